# revision 11
# baseline (speedup 1.0000x reference)
"""Self-contained Trainium2 Bass kernel for nn_CrossLayerBlock (MoE routing).

8-way token parallelism; uniform SPMD program via per-core token permutation
(own tokens first). Causality = data-driven key-bias rows folded into the score
matmul + 4 universal diagonal mask tiles. Router in true fp32; big matmuls in
float32r; attention probabilities/V in bf16. MoE: global capacity via counts
AllGather + prefix-sum matmuls; kept rows scattered per-expert (indirect DMA),
dense per-expert MLP on compacted buffers, indirect gather back.
"""
import numpy as np
from contextlib import ExitStack

import concourse.bass as bass
import concourse.tile as tile
from concourse import bacc, mybir
from concourse import bass_utils
from concourse.masks import make_identity

B, T, D, H, HS, E = 4, 2048, 768, 12, 64, 8
NCORES = 8
NOWN = 1024
NKC = 16
DC = 6
SE = 256
ZROW = E * SE
LN_EPS = 1e-5
NEG = -30.0

f32 = mybir.dt.float32
f32r = mybir.dt.float32r
bf16 = mybir.dt.bfloat16
i32 = mybir.dt.int32
u32 = mybir.dt.uint32
AF = mybir.ActivationFunctionType
ALU = mybir.AluOpType

KCS0 = [8, 9, 10, 11, 0, 1, 2, 3]
KCS1 = [8, 9, 10, 11, 12, 13, 14, 15, 0, 1, 2, 3, 4, 5, 6, 7]


def _ln(nc, pool, xt, eps_col):
    r = pool.tile([128, 1], f32, tag="ln_r", name="ln_r")
    nc.vector.reduce_sum(r[:], xt[:], axis=mybir.AxisListType.X)
    mu = pool.tile([128, 1], f32, tag="ln_mu", name="ln_mu")
    nc.vector.tensor_scalar_mul(mu[:], r[:], 1.0 / D)
    xc = pool.tile([128, D], f32, tag="ln_xc", name="ln_xc")
    nc.vector.tensor_scalar(xc[:], xt[:], mu[:, :1], None, ALU.subtract)
    sq = pool.tile([128, D], f32, tag="ln_sq", name="ln_sq")
    nc.scalar.square(sq[:], xc[:])
    v = pool.tile([128, 1], f32, tag="ln_v", name="ln_v")
    nc.vector.reduce_sum(v[:], sq[:], axis=mybir.AxisListType.X)
    nc.vector.tensor_scalar_mul(v[:], v[:], 1.0 / D)
    lnv = pool.tile([128, 1], f32, tag="ln_lnv", name="ln_lnv")
    nc.scalar.activation(lnv[:], v[:], AF.Ln, bias=eps_col[:, :1])
    rstd = pool.tile([128, 1], f32, tag="ln_rstd", name="ln_rstd")
    nc.scalar.activation(rstd[:], lnv[:], AF.Exp, scale=-0.5)
    xn = pool.tile([128, D], f32, tag="ln_xn", name="ln_xn")
    nc.vector.tensor_scalar(xn[:], xc[:], rstd[:, :1], None, ALU.mult)
    return xn


def build_program():
    nc = bacc.Bacc("TRN2", target_bir_lowering=False, debug=False,
                   enable_asserts=False, num_devices=NCORES)

    din = {}
    for name, shape, dt in [
        ("xp", [T, D], f32), ("noise", [NOWN, E], f32),
        ("wq", [D, D], f32), ("wk", [D, D], f32), ("wv", [D, D], f32),
        ("wo", [D, D], f32), ("wrout", [D, 17], f32), ("rbias", [1, 17], f32),
        ("we1", [E, D, 4 * D], f32), ("we2", [E, 4 * D, D], f32),
        ("kbias", [2, T], f32), ("qsel", [2, NOWN], f32),
        ("chunksel", [64, E], f32),
    ]:
        din[name] = nc.dram_tensor(name, shape, dt, kind="ExternalInput").ap()

    yout = nc.dram_tensor("yout", [NOWN, D], f32, kind="ExternalOutput").ap()
    x1dbg = nc.dram_tensor("x1dbg", [NOWN, D], f32, kind="ExternalOutput").ap()
    rdbg = nc.dram_tensor("rdbg", [NOWN, E], f32, kind="ExternalOutput").ap()

    with tile.TileContext(nc) as tc, ExitStack() as top:
        dram = top.enter_context(tc.tile_pool(name="dram", bufs=1, space="DRAM"))
        xe_dram = dram.tile([E * SE, D], f32)
        ye_dram = dram.tile([E * SE + 1, D], f32)
        cc_in = dram.tile([9, 8], f32)
        cc_out = dram.tile([72, 8], f32, addr_space="Shared")

        const = top.enter_context(tc.tile_pool(name="const", bufs=1))
        ident = const.tile([128, 128], f32)
        make_identity(nc, ident[:])
        lincl = const.tile([128, 128], f32)
        nc.gpsimd.memset(lincl[:], 1.0)
        nc.gpsimd.affine_select(out=lincl[:], in_=lincl[:],
                                compare_op=ALU.is_ge, fill=0.0, base=0,
                                pattern=[[1, 128]], channel_multiplier=-1)
        lstrict = const.tile([128, 128], f32)
        nc.gpsimd.memset(lstrict[:], 1.0)
        nc.gpsimd.affine_select(out=lstrict[:], in_=lstrict[:],
                                compare_op=ALU.is_gt, fill=0.0, base=0,
                                pattern=[[1, 128]], channel_multiplier=-1)
        dm = []
        for d in range(4):
            dmf = const.tile([128, 512], f32, tag=f"dmf_{d}", name=f"dmf_{d}")
            nc.gpsimd.memset(dmf[:], 1.0)
            nc.gpsimd.affine_select(out=dmf[:], in_=dmf[:],
                                    compare_op=ALU.is_ge, fill=0.0,
                                    base=-d * 128, pattern=[[1, 512]],
                                    channel_multiplier=-1)
            dmb = const.tile([128, 512], bf16, tag=f"dmb_{d}", name=f"dmb_{d}")
            nc.vector.tensor_copy(dmb[:], dmf[:])
            dm.append(dmb)
        ones_r = const.tile([128, 1], f32)
        nc.vector.memset(ones_r[:], 1.0)
        ones1r = const.tile([1, 128], f32)
        nc.vector.memset(ones1r[:], 1.0)
        iota8i = const.tile([128, 8], i32)
        nc.gpsimd.iota(iota8i[:], pattern=[[1, 8]], base=0, channel_multiplier=0)
        iota8 = const.tile([128, 8], f32)
        nc.vector.tensor_copy(iota8[:], iota8i[:])
        iotase = const.tile([128, 8], f32)
        nc.vector.tensor_scalar_mul(iotase[:], iota8[:], float(SE))
        eps_col = const.tile([128, 1], f32)
        nc.vector.memset(eps_col[:], LN_EPS)
        rbias_bc = const.tile([128, 17], f32)
        rb1 = const.tile([1, 17], f32)
        nc.sync.dma_start(rb1[:], din["rbias"][:])
        nc.gpsimd.partition_broadcast(rbias_bc[:], rb1[:])
        wrout_sb = const.tile([128, DC, 17], f32)
        nc.sync.dma_start(wrout_sb[:],
                          din["wrout"].rearrange("(a p) n -> p a n", p=128))
        chsel_sb = const.tile([64, E], f32)
        nc.sync.dma_start(chsel_sb[:], din["chunksel"][:])

        x1_t, xn2_t = [], []
        g1_t, g2_t, ns_t, m0_t, m1_t, m_t, maug_t, gf_t = ([] for _ in range(8))
        keep_t, keepr_t, gidx_t = [], [], []

        with ExitStack() as sCF:
            if True:
                pbcp = sCF.enter_context(tc.tile_pool(name="pbcp", bufs=1))
                attT = pbcp.tile([128, DC, NOWN], f32r)
                xn2T = pbcp.tile([128, DC, NOWN], f32)
                with ExitStack() as sAB:
                    pab = sAB.enter_context(tc.tile_pool(name="pab", bufs=1))
                    xnT = pab.tile([128, DC, T], f32r)
                    vaug = pab.tile([128, NKC, H, HS + 1], bf16)

                    # ---------- Phase A ----------
                    with ExitStack() as sA:
                        wvp = sA.enter_context(tc.tile_pool(name="wvp", bufs=1))
                        wv_sb = wvp.tile([128, DC, D], f32r)
                        nc.sync.dma_start(
                            wv_sb[:],
                            din["wv"].rearrange("(a p) n -> p a n",
                                                p=128).bitcast(f32r))
                        apool = sA.enter_context(tc.tile_pool(name="pa_sb",
                                                              bufs=2))
                        aps = sA.enter_context(
                            tc.tile_pool(name="pa_ps", bufs=2, space="PSUM"))
                        for kc in range(NKC):
                            xt = apool.tile([128, D], f32, tag="xt", name="xt")
                            nc.sync.dma_start(
                                xt[:], din["xp"][kc * 128:(kc + 1) * 128, :])
                            xn = _ln(nc, apool, xt, eps_col)
                            for dc in range(DC):
                                tp = aps.tile([128, 128], f32, tag="tp",
                                              name="tp")
                                nc.tensor.transpose(
                                    tp[:], xn[:, dc * 128:(dc + 1) * 128],
                                    ident[:])
                                nc.vector.tensor_copy(
                                    xnT[:, dc, kc * 128:(kc + 1) * 128], tp[:])
                            for nb in range(2):
                                vp = aps.tile([128, 384], f32, tag="vp",
                                              name="vp")
                                for dc in range(DC):
                                    nc.tensor.matmul(
                                        vp[:],
                                        xnT[:, dc, kc * 128:(kc + 1) * 128],
                                        wv_sb[:, dc, nb * 384:(nb + 1) * 384],
                                        start=(dc == 0), stop=(dc == DC - 1))
                                nc.vector.tensor_copy(
                                    vaug[:, kc, nb * 6:(nb + 1) * 6, 0:HS],
                                    vp[:].rearrange("p (h e) -> p h e", e=HS))
                        nc.vector.memset(vaug[:, :, :, HS:HS + 1], 1.0)

                    # ---------- Phase B ----------
                    with ExitStack() as sB:
                        wqk = sB.enter_context(tc.tile_pool(name="wqk", bufs=1))
                        wq_sb = wqk.tile([128, DC, D], f32r, tag="wq_sb",
                                         name="wq_sb")
                        nc.sync.dma_start(
                            wq_sb[:],
                            din["wq"].rearrange("(a p) n -> p a n",
                                                p=128).bitcast(f32r))
                        wk_sb = wqk.tile([128, DC, D], f32r, tag="wk_sb",
                                         name="wk_sb")
                        nc.sync.dma_start(
                            wk_sb[:],
                            din["wk"].rearrange("(a p) n -> p a n",
                                                p=128).bitcast(f32r))
                        kqpool = sB.enter_context(tc.tile_pool(name="pb_kq",
                                                               bufs=1))
                        bpool = sB.enter_context(tc.tile_pool(name="pb_sb",
                                                              bufs=3))
                        bps = sB.enter_context(
                            tc.tile_pool(name="pb_ps", bufs=2, space="PSUM"))
                        atps = sB.enter_context(
                            tc.tile_pool(name="pb_at", bufs=2, space="PSUM"))
                        for h in range(H):
                            kT = kqpool.tile([66, T], f32r, tag="kT", name="kT")
                            nc.sync.dma_start(kT[64:66, :],
                                              din["kbias"][:].bitcast(f32r))
                            for qb in range(4):
                                kp = bps.tile([64, 512], f32, tag="kp",
                                              name="kp")
                                for dc in range(DC):
                                    nc.tensor.matmul(
                                        kp[:],
                                        wk_sb[:, dc, h * 64:(h + 1) * 64],
                                        xnT[:, dc, qb * 512:(qb + 1) * 512],
                                        start=(dc == 0), stop=(dc == DC - 1))
                                nc.vector.tensor_copy(
                                    kT[0:64, qb * 512:(qb + 1) * 512], kp[:])
                            qT = kqpool.tile([66, NOWN], f32r, tag="qT",
                                             name="qT")
                            nc.sync.dma_start(qT[64:66, :],
                                              din["qsel"][:].bitcast(f32r))
                            for qb in range(2):
                                qp = bps.tile([64, 512], f32, tag="kp",
                                              name="qp")
                                for dc in range(DC):
                                    nc.tensor.matmul(
                                        qp[:],
                                        wq_sb[:, dc, h * 64:(h + 1) * 64],
                                        xnT[:, dc, qb * 512:(qb + 1) * 512],
                                        start=(dc == 0), stop=(dc == DC - 1))
                                nc.vector.tensor_copy(
                                    qT[0:64, qb * 512:(qb + 1) * 512], qp[:])

                            for slot, kcs in ((0, KCS0), (1, KCS1)):
                                at = atps.tile([65, 512], f32, tag="at",
                                               name="at")
                                for j, kc in enumerate(kcs):
                                    st = bps.tile([128, 512], f32, tag="st",
                                                  name="st")
                                    nc.tensor.matmul(
                                        st[:], kT[:, kc * 128:(kc + 1) * 128],
                                        qT[:, slot * 512:(slot + 1) * 512],
                                        start=True, stop=True)
                                    exr = bpool.tile([128, 512], bf16,
                                                     tag="exr", name="exr")
                                    nc.scalar.activation(exr[:], st[:], AF.Exp)
                                    if kc < 8 and kc // 4 == slot:
                                        nc.vector.tensor_tensor(
                                            out=exr[:], in0=exr[:],
                                            in1=dm[kc % 4][:], op=ALU.mult)
                                    nc.tensor.matmul(
                                        at[:], vaug[:, kc, h, :], exr[:],
                                        start=(j == 0),
                                        stop=(j == len(kcs) - 1))
                                rec = bpool.tile([1, 512], f32, tag="rec",
                                                 name="rec")
                                nc.vector.reciprocal(rec[:], at[64:65, :])
                                pbt = bpool.tile([64, 512], f32, tag="pbt",
                                                 name="pbt")
                                nc.gpsimd.partition_broadcast(pbt[:], rec[:])
                                nc.vector.tensor_tensor(
                                    out=attT[(h % 2) * 64:(h % 2) * 64 + 64,
                                             h // 2,
                                             slot * 512:(slot + 1) * 512],
                                    in0=at[0:64, :], in1=pbt[:], op=ALU.mult)

                # ---------- Phase C ----------
                cpersist = top.enter_context(tc.tile_pool(name="cpersist",
                                                          bufs=1,
                                                          side="right"))
                with ExitStack() as sC:
                    wop = sC.enter_context(tc.tile_pool(name="wop", bufs=1))
                    wo_sb = wop.tile([128, DC, D], f32r)
                    nc.sync.dma_start(
                        wo_sb[:],
                        din["wo"].rearrange("(a p) n -> p a n",
                                            p=128).bitcast(f32r))
                    ctmp = sC.enter_context(tc.tile_pool(name="ctmp", bufs=2))
                    cps = sC.enter_context(
                        tc.tile_pool(name="pc_ps", bufs=2, space="PSUM"))
                    for tt in range(8):
                        xo = ctmp.tile([128, D], f32, tag="xo", name="xo")
                        nc.sync.dma_start(
                            xo[:], din["xp"][tt * 128:(tt + 1) * 128, :])
                        x1 = cpersist.tile([128, D], f32, tag=f"x1_{tt}",
                                           name=f"x1_{tt}")
                        for nb in range(2):
                            yp = cps.tile([128, 384], f32, tag="yp", name="yp")
                            for dc in range(DC):
                                nc.tensor.matmul(
                                    yp[:],
                                    attT[:, dc, tt * 128:(tt + 1) * 128],
                                    wo_sb[:, dc, nb * 384:(nb + 1) * 384],
                                    start=(dc == 0), stop=(dc == DC - 1))
                            nc.vector.tensor_add(
                                x1[:, nb * 384:(nb + 1) * 384], yp[:],
                                xo[:, nb * 384:(nb + 1) * 384])
                        nc.sync.dma_start(x1dbg[tt * 128:(tt + 1) * 128, :],
                                          x1[:])
                        xn2raw = _ln(nc, ctmp, x1, eps_col)
                        xn2 = cpersist.tile([128, D], f32, tag=f"xn2_{tt}",
                                            name=f"xn2_{tt}")
                        nc.vector.tensor_copy(xn2[:], xn2raw[:])
                        for dc in range(DC):
                            tp = cps.tile([128, 128], f32, tag="tp2",
                                          name="tp2")
                            nc.tensor.transpose(
                                tp[:], xn2[:, dc * 128:(dc + 1) * 128],
                                ident[:])
                            nc.vector.tensor_copy(
                                xn2T[:, dc, tt * 128:(tt + 1) * 128], tp[:])
                        x1_t.append(x1)
                        xn2_t.append(xn2)

            # ---------- Phase D: router ----------
            rpool = top.enter_context(tc.tile_pool(name="rpool", bufs=1,
                                                   side="right"))
            rps = sCF.enter_context(tc.tile_pool(name="pd_ps", bufs=1,
                                                 space="PSUM"))
            cnt_ps = rps.tile([9, 8], f32)
            with ExitStack() as pd:
                dps = pd.enter_context(
                    tc.tile_pool(name="pd_ps2", bufs=2, space="PSUM"))
                dpool = pd.enter_context(tc.tile_pool(name="pd_tmp", bufs=2))
                for tt in range(8):
                    rp = dps.tile([128, 17], f32, tag="rp", name="rp")
                    for dc in range(DC):
                        nc.tensor.matmul(
                            rp[:], xn2T[:, dc, tt * 128:(tt + 1) * 128],
                            wrout_sb[:, dc, :],
                            start=(dc == 0), stop=(dc == DC - 1))
                    rt = dpool.tile([128, 17], f32, tag="rt", name="rt")
                    nc.vector.tensor_add(rt[:], rp[:], rbias_bc[:])
                    z = dpool.tile([128, 8], f32, tag="z", name="z")
                    nc.scalar.activation(z[:], rt[:, 8:16], AF.Exp)
                    zp1 = dpool.tile([128, 8], f32, tag="zp1", name="zp1")
                    nc.vector.tensor_scalar_add(zp1[:], z[:], 1.0)
                    sp0 = dpool.tile([128, 8], f32, tag="sp0", name="sp0")
                    nc.scalar.activation(sp0[:], zp1[:], AF.Ln)
                    en = dpool.tile([128, 8], f32, tag="en", name="en")
                    nc.scalar.activation(en[:], sp0[:], AF.Exp, scale=-1.0)
                    t1 = dpool.tile([128, 8], f32, tag="t1", name="t1")
                    nc.vector.tensor_tensor(out=t1[:], in0=zp1[:], in1=en[:],
                                            op=ALU.mult)
                    nc.vector.tensor_scalar_add(t1[:], t1[:], -1.0)
                    sp = dpool.tile([128, 8], f32, tag="sp", name="sp")
                    nc.vector.tensor_add(sp[:], sp0[:], t1[:])
                    nt = dpool.tile([128, 8], f32, tag="nt", name="nt")
                    nc.sync.dma_start(
                        nt[:], din["noise"][tt * 128:(tt + 1) * 128, :])
                    nm = dpool.tile([128, 8], f32, tag="nm", name="nm")
                    nc.vector.tensor_tensor(out=nm[:], in0=nt[:], in1=sp[:],
                                            op=ALU.mult)
                    noisy = dpool.tile([128, 8], f32, tag="noisy", name="noisy")
                    nc.vector.tensor_add(noisy[:], rt[:, 0:8], nm[:])
                    t8 = dpool.tile([128, 8], f32, tag="t8", name="t8")
                    nc.vector.max(t8[:], noisy[:])
                    ix = dpool.tile([128, 8], u32, tag="ix", name="ix")
                    nc.vector.max_index(ix[:], t8[:], noisy[:])
                    ixf = dpool.tile([128, 8], f32, tag="ixf", name="ixf")
                    nc.vector.tensor_copy(ixf[:], ix[:])
                    dv = dpool.tile([128, 1], f32, tag="dv", name="dv")
                    nc.vector.tensor_sub(dv[:], t8[:, 1:2], t8[:, 0:1])
                    ge = dpool.tile([128, 1], f32, tag="ge", name="ge")
                    nc.scalar.activation(ge[:], dv[:], AF.Exp)
                    gp1 = dpool.tile([128, 1], f32, tag="gp1", name="gp1")
                    nc.vector.tensor_scalar_add(gp1[:], ge[:], 1.0)
                    g1 = rpool.tile([128, 1], f32, tag=f"g1_{tt}",
                                    name=f"g1_{tt}")
                    nc.vector.reciprocal(g1[:], gp1[:])
                    g2 = rpool.tile([128, 1], f32, tag=f"g2_{tt}",
                                    name=f"g2_{tt}")
                    nc.vector.tensor_tensor(out=g2[:], in0=ge[:], in1=g1[:],
                                            op=ALU.mult)
                    ns = rpool.tile([128, 1], f32, tag=f"ns_{tt}",
                                    name=f"ns_{tt}")
                    nc.vector.tensor_scalar(ns[:], rt[:, 16:17], 0.0, None,
                                            ALU.is_le)
                    m0 = rpool.tile([128, 8], f32, tag=f"m0_{tt}",
                                    name=f"m0_{tt}")
                    nc.vector.tensor_scalar(m0[:], iota8[:], ixf[:, 0:1], None,
                                            ALU.is_equal)
                    m1 = rpool.tile([128, 8], f32, tag=f"m1_{tt}",
                                    name=f"m1_{tt}")
                    nc.vector.tensor_scalar(m1[:], iota8[:], ixf[:, 1:2], None,
                                            ALU.is_equal)
                    gf = rpool.tile([128, 8], f32, tag=f"gf_{tt}",
                                    name=f"gf_{tt}")
                    ga = dpool.tile([128, 8], f32, tag="ga", name="ga")
                    nc.vector.tensor_scalar(ga[:], m0[:], g1[:, :1], None,
                                            ALU.mult)
                    gb = dpool.tile([128, 8], f32, tag="gb", name="gb")
                    nc.vector.tensor_scalar(gb[:], m1[:], g2[:, :1], None,
                                            ALU.mult)
                    nc.vector.tensor_add(gf[:], ga[:], gb[:])
                    m = rpool.tile([128, 8], f32, tag=f"m_{tt}", name=f"m_{tt}")
                    nc.vector.tensor_add(m[:], m0[:], m1[:])
                    nc.vector.tensor_scalar_min(m[:], m[:], 1.0)
                    nc.vector.tensor_scalar(m[:], m[:], ns[:, :1], None,
                                            ALU.mult)
                    maug = rpool.tile([128, 9], f32, tag=f"maug_{tt}",
                                      name=f"maug_{tt}")
                    nc.vector.tensor_copy(maug[:, 0:8], m[:])
                    nc.vector.tensor_copy(maug[:, 8:9], ns[:])
                    nc.tensor.matmul(cnt_ps[:, tt:tt + 1], maug[:], ones_r[:],
                                     start=True, stop=True)
                    g1_t.append(g1); g2_t.append(g2); ns_t.append(ns)
                    m0_t.append(m0); m1_t.append(m1); m_t.append(m)
                    maug_t.append(maug); gf_t.append(gf)

            cnt_sb = rpool.tile([9, 8], f32)
            nc.vector.tensor_copy(cnt_sb[:], cnt_ps[:])
            nc.sync.dma_start(cc_in[:], cnt_sb[:])
            nc.gpsimd.collective_compute(
                "AllGather", ALU.bypass, replica_groups=[list(range(NCORES))],
                ins=[cc_in.opt()], outs=[cc_out.opt()])
            cnts_all = rpool.tile([72, 8], f32)
            nc.sync.dma_start(cnts_all[:], cc_out[:])

            # ---------- Phase E ----------
            flat = rpool.tile([9, 64], f32)
            for r in range(NCORES):
                b2, a = r // 2, r % 2
                if a == 0:
                    nc.sync.dma_start(flat[:, b2 * 16:b2 * 16 + 4],
                                      cnts_all[9 * r:9 * r + 9, 0:4])
                    nc.sync.dma_start(flat[:, b2 * 16 + 12:b2 * 16 + 16],
                                      cnts_all[9 * r:9 * r + 9, 4:8])
                else:
                    nc.sync.dma_start(flat[:, b2 * 16 + 4:b2 * 16 + 12],
                                      cnts_all[9 * r:9 * r + 9, 0:8])
            zf = rpool.tile([9, 64], f32)
            nc.vector.memset(zf[:], 0.0)
            incl = rpool.tile([9, 64], f32)
            nc.vector.tensor_tensor_scan(incl[:], flat[:], zf[:], 0.0,
                                         ALU.add, ALU.add)
            excl = rpool.tile([9, 64], f32)
            nc.vector.tensor_sub(excl[:], incl[:], flat[:])
            tot = rpool.tile([1, 1], f32)
            nc.sync.dma_start(tot[:], incl[8:9, 63:64])
            tot_i = rpool.tile([1, 1], i32)
            nc.vector.tensor_copy(tot_i[:], tot[:])
            cap_i = rpool.tile([1, 1], i32)
            nc.vector.tensor_scalar(cap_i[:], tot_i[:], 2, None,
                                    ALU.arith_shift_right)
            capt = rpool.tile([1, 1], f32)
            nc.vector.tensor_copy(capt[:], cap_i[:])
            cap_bc = rpool.tile([128, 1], f32)
            nc.gpsimd.partition_broadcast(cap_bc[:], capt[:])

            exT_ps = rps.tile([64, 9], f32)
            nc.tensor.transpose(exT_ps[:], excl[:, 0:64], ident[0:9, 0:9])
            exT = rpool.tile([64, 9], f32)
            nc.vector.tensor_copy(exT[:], exT_ps[:])
            myo_ps = rps.tile([9, 8], f32)
            nc.tensor.matmul(myo_ps[:], exT[:, 0:9], chsel_sb[:], start=True,
                             stop=True)
            myo = rpool.tile([9, 8], f32)
            nc.vector.tensor_copy(myo[:], myo_ps[:])
            myoT_ps = rps.tile([8, 9], f32)
            nc.tensor.transpose(myoT_ps[:], myo[:], ident[0:9, 0:9])
            myoT = rpool.tile([8, 9], f32)
            nc.vector.tensor_copy(myoT[:], myoT_ps[:])

            # ---------- Phase F ----------
            kcnt_ps = rps.tile([8, 8], f32)
            with ExitStack() as pf:
                fps = pf.enter_context(
                    tc.tile_pool(name="pf_ps", bufs=2, space="PSUM"))
                for tt in range(8):
                    pr = fps.tile([128, 8], f32, tag="pr", name="pr")
                    orow = rpool.tile([1, 8], f32, tag=f"orow_{tt}",
                                      name=f"orow_{tt}")
                    nc.sync.dma_start(orow[:], myoT[tt:tt + 1, 0:8])
                    nc.tensor.matmul(pr[:], ones1r[:], orow[:],
                                     start=True, stop=False)
                    nc.tensor.matmul(pr[:], lincl[:], maug_t[tt][:, 0:8],
                                     start=False, stop=True)
                    keepb = rpool.tile([128, 8], f32, tag=f"kb_{tt}",
                                       name=f"kb_{tt}")
                    nc.vector.tensor_scalar(keepb[:], pr[:], cap_bc[:, :1],
                                            None, ALU.is_le)
                    keep = rpool.tile([128, 8], f32, tag=f"keep_{tt}",
                                      name=f"keep_{tt}")
                    nc.vector.tensor_tensor(out=keep[:], in0=keepb[:],
                                            in1=m_t[tt][:], op=ALU.mult)
                    nc.tensor.matmul(kcnt_ps[:, tt:tt + 1], keep[:],
                                     ones_r[:], start=True, stop=True)
                    kg = rpool.tile([128, 8], f32, tag=f"kg_{tt}",
                                    name=f"kg_{tt}")
                    nc.vector.tensor_tensor(out=kg[:], in0=keep[:],
                                            in1=gf_t[tt][:], op=ALU.mult)
                    nc.sync.dma_start(rdbg[tt * 128:(tt + 1) * 128, :], kg[:])
                    keep_t.append(keep); keepr_t.append(keep)

            kcnt = rpool.tile([8, 8], f32)
            nc.vector.tensor_copy(kcnt[:], kcnt_ps[:])
            zf8 = rpool.tile([8, 8], f32)
            nc.vector.memset(zf8[:], 0.0)
            kincl = rpool.tile([8, 8], f32)
            nc.vector.tensor_tensor_scan(kincl[:], kcnt[:], zf8[:], 0.0,
                                         ALU.add, ALU.add)
            kexcl = rpool.tile([8, 8], f32)
            nc.vector.tensor_sub(kexcl[:], kincl[:], kcnt[:])
            kexT_ps = rps.tile([8, 8], f32)
            nc.tensor.transpose(kexT_ps[:], kexcl[:], ident[0:8, 0:8])
            kexT = rpool.tile([8, 8], f32)
            nc.vector.tensor_copy(kexT[:], kexT_ps[:])

            with ExitStack() as pf2:
                f2ps = pf2.enter_context(
                    tc.tile_pool(name="pf2_ps", bufs=2, space="PSUM"))
                f2p = pf2.enter_context(tc.tile_pool(name="pf2_sb", bufs=2))
                for tt in range(8):
                    p2 = f2ps.tile([128, 8], f32, tag="p2", name="p2")
                    krow = rpool.tile([1, 8], f32, tag=f"krow_{tt}",
                                      name=f"krow_{tt}")
                    nc.sync.dma_start(krow[:], kexT[tt:tt + 1, :])
                    nc.tensor.matmul(p2[:], ones1r[:], krow[:],
                                     start=True, stop=False)
                    nc.tensor.matmul(p2[:], lstrict[:], keepr_t[tt][:],
                                     start=False, stop=True)
                    slotf = f2p.tile([128, 8], f32, tag="slotf", name="slotf")
                    nc.vector.tensor_add(slotf[:], p2[:], iotase[:])
                    gidx = rpool.tile([128, 2], i32, tag=f"gi_{tt}",
                                      name=f"gi_{tt}")
                    for k, mk in ((0, m0_t[tt]), (1, m1_t[tt])):
                        fim = f2p.tile([128, 8], f32, tag="fim", name="fim")
                        nc.vector.tensor_tensor(out=fim[:], in0=slotf[:],
                                                in1=mk[:], op=ALU.mult)
                        fi = f2p.tile([128, 1], f32, tag="fi", name="fi")
                        nc.vector.reduce_sum(fi[:], fim[:],
                                             axis=mybir.AxisListType.X)
                        km = f2p.tile([128, 8], f32, tag="km", name="km")
                        nc.vector.tensor_tensor(out=km[:], in0=mk[:],
                                                in1=keep_t[tt][:], op=ALU.mult)
                        kept = f2p.tile([128, 1], f32, tag="kept", name="kept")
                        nc.vector.reduce_sum(kept[:], km[:],
                                             axis=mybir.AxisListType.X)
                        u = f2p.tile([128, 1], f32, tag="u", name="u")
                        nc.vector.tensor_scalar_add(u[:], kept[:], -1.0)
                        nc.vector.tensor_scalar_mul(u[:], u[:], -70000.0)
                        fis = f2p.tile([128, 1], f32, tag="fis", name="fis")
                        nc.vector.tensor_add(fis[:], fi[:], u[:])
                        fii = f2p.tile([128, 1], i32, tag="fii", name="fii")
                        nc.vector.tensor_copy(fii[:], fis[:])
                        nc.gpsimd.indirect_dma_start(
                            out=xe_dram[:],
                            out_offset=bass.IndirectOffsetOnAxis(
                                ap=fii[:, :1], axis=0),
                            in_=xn2_t[tt][:], in_offset=None,
                            bounds_check=E * SE - 1, oob_is_err=False)
                        gi = f2p.tile([128, 1], f32, tag="gi2", name="gi2")
                        nc.vector.tensor_tensor(out=gi[:], in0=fi[:],
                                                in1=kept[:], op=ALU.mult)
                        w = f2p.tile([128, 1], f32, tag="u2", name="u2")
                        nc.vector.tensor_scalar_add(w[:], kept[:], -1.0)
                        nc.vector.tensor_scalar_mul(w[:], w[:], -float(ZROW))
                        nc.vector.tensor_add(gi[:], gi[:], w[:])
                        nc.vector.tensor_copy(gidx[:, k:k + 1], gi[:])
                    gidx_t.append(gidx)

        # ---------- Phase G: expert MLPs ----------
        zrow = rpool.tile([128, D], f32)
        nc.vector.memset(zrow[:], 0.0)
        nc.sync.dma_start(ye_dram[ZROW:ZROW + 1, :], zrow[0:1, :])
        with ExitStack() as pg:
            gsb = pg.enter_context(tc.tile_pool(name="pg_sb", bufs=2))
            w1p = pg.enter_context(tc.tile_pool(name="pg_w1", bufs=3))
            w2p = pg.enter_context(tc.tile_pool(name="pg_w2", bufs=3))
            hpool = pg.enter_context(tc.tile_pool(name="pg_h", bufs=2))
            gps = pg.enter_context(
                tc.tile_pool(name="pg_ps", bufs=2, space="PSUM"))
            yps = pg.enter_context(
                tc.tile_pool(name="pg_yps", bufs=1, space="PSUM"))
            for e in range(E):
                xeT = gsb.tile([128, DC, SE], f32r, tag="xeT", name="xeT")
                for i2 in range(SE // 128):
                    xe = gsb.tile([128, D], f32, tag="xe", name="xe")
                    nc.sync.dma_start(
                        xe[:],
                        xe_dram[e * SE + i2 * 128:e * SE + (i2 + 1) * 128, :])
                    for dc in range(DC):
                        tp = gps.tile([128, 128], f32, tag="tp3", name="tp3")
                        nc.tensor.transpose(
                            tp[:], xe[:, dc * 128:(dc + 1) * 128], ident[:])
                        nc.vector.tensor_copy(
                            xeT[:, dc, i2 * 128:(i2 + 1) * 128], tp[:])
                hT = hpool.tile([128, 24, SE], f32r, tag="hT", name="hT")
                for mt in range(24):
                    w1t = w1p.tile([128, DC, 128], f32r, tag="w1t", name="w1t")
                    nc.sync.dma_start(
                        w1t[:],
                        din["we1"][e].rearrange("(a p) n -> p a n", p=128)
                        [:, :, mt * 128:(mt + 1) * 128].bitcast(f32r))
                    hp = gps.tile([128, SE], f32, tag="hp", name="hp")
                    for dc in range(DC):
                        nc.tensor.matmul(hp[:], w1t[:, dc, :], xeT[:, dc, :],
                                         start=(dc == 0), stop=(dc == DC - 1))
                    nc.vector.tensor_scalar_max(hT[:, mt, :], hp[:], 0.0)
                ypl = [yps.tile([128, 384], f32, tag=f"yp_{i}",
                                name=f"ypl_{i}") for i in range(4)]
                for hc in range(24):
                    w2t = w2p.tile([128, D], f32r, tag="w2t", name="w2t")
                    nc.sync.dma_start(
                        w2t[:],
                        din["we2"][e][hc * 128:(hc + 1) * 128, :].bitcast(f32r))
                    for rt2 in range(2):
                        for nb in range(2):
                            nc.tensor.matmul(
                                ypl[rt2 * 2 + nb][:],
                                hT[:, hc, rt2 * 128:(rt2 + 1) * 128],
                                w2t[:, nb * 384:(nb + 1) * 384],
                                start=(hc == 0), stop=(hc == 23))
                for rt2 in range(2):
                    ysb = gsb.tile([128, D], f32, tag="ysb", name="ysb")
                    for nb in range(2):
                        nc.scalar.copy(ysb[:, nb * 384:(nb + 1) * 384],
                                       ypl[rt2 * 2 + nb][:])
                    nc.sync.dma_start(
                        ye_dram[e * SE + rt2 * 128:
                                e * SE + (rt2 + 1) * 128, :], ysb[:])

        # ---------- Phase H ----------
        with ExitStack() as ph:
            hsb = ph.enter_context(tc.tile_pool(name="ph_sb", bufs=3))
            for tt in range(8):
                yg0 = hsb.tile([128, D], f32, tag="yg0", name="yg0")
                nc.gpsimd.indirect_dma_start(
                    out=yg0[:], out_offset=None, in_=ye_dram[:],
                    in_offset=bass.IndirectOffsetOnAxis(
                        ap=gidx_t[tt][:, 0:1], axis=0))
                yg1 = hsb.tile([128, D], f32, tag="yg1", name="yg1")
                nc.gpsimd.indirect_dma_start(
                    out=yg1[:], out_offset=None, in_=ye_dram[:],
                    in_offset=bass.IndirectOffsetOnAxis(
                        ap=gidx_t[tt][:, 1:2], axis=0))
                u0 = hsb.tile([128, D], f32, tag="u0", name="u0")
                nc.vector.tensor_scalar(u0[:], yg0[:], g1_t[tt][:, :1], None,
                                        ALU.mult)
                u1 = hsb.tile([128, D], f32, tag="u1", name="u1")
                nc.vector.tensor_scalar(u1[:], yg1[:], g2_t[tt][:, :1], None,
                                        ALU.mult)
                upd = hsb.tile([128, D], f32, tag="upd", name="upd")
                nc.vector.tensor_add(upd[:], u0[:], u1[:])
                nc.vector.tensor_sub(upd[:], upd[:], xn2_t[tt][:])
                nc.vector.tensor_scalar(upd[:], upd[:], ns_t[tt][:, :1], None,
                                        ALU.mult)
                nc.vector.tensor_add(upd[:], upd[:], xn2_t[tt][:])
                out = hsb.tile([128, D], f32, tag="out", name="out")
                nc.vector.tensor_add(out[:], x1_t[tt][:], upd[:])
                nc.sync.dma_start(yout[tt * 128:(tt + 1) * 128, :], out[:])

    nc.compile()
    return nc


_OWN = {0: [0, 1, 2, 3, 12, 13, 14, 15], 1: [4, 5, 6, 7, 8, 9, 10, 11]}


def _core_meta(c):
    b, a = c // 2, c % 2
    own = _OWN[a]
    other = [g for g in range(16) if g not in own]
    perm_chunks = own + other
    rows = np.concatenate([np.arange(g * 128, (g + 1) * 128)
                           for g in perm_chunks])
    return b, a, own, rows


def _host_inputs(x, noise, Wq, Wk, Wv, Wo, Wr, br, Wn, bn, Wsk, bsk, We1, We2,
                 **_unused):
    x = np.asarray(x, np.float32)
    noise = np.asarray(noise, np.float32)
    wq = np.ascontiguousarray(
        (np.transpose(np.asarray(Wq), (1, 0, 2)).reshape(D, D)
         * np.float32(D ** -0.5)).astype(np.float32))
    wk = np.ascontiguousarray(
        np.transpose(np.asarray(Wk), (1, 0, 2)).reshape(D, D)
        .astype(np.float32))
    wv = np.ascontiguousarray(
        np.transpose(np.asarray(Wv), (1, 0, 2)).reshape(D, D)
        .astype(np.float32))
    wrout = np.ascontiguousarray(np.concatenate(
        [np.asarray(Wr), np.asarray(Wn), np.asarray(Wsk)], axis=1)
        .astype(np.float32))
    rbias = np.concatenate(
        [np.asarray(br), np.asarray(bn), np.asarray(bsk)])[None, :] \
        .astype(np.float32)
    qsel = np.zeros((2, NOWN), np.float32)
    qsel[0, 0:512] = 1.0
    qsel[1, 512:1024] = 1.0
    we1 = np.ascontiguousarray(np.asarray(We1, np.float32))
    we2 = np.ascontiguousarray(np.asarray(We2, np.float32))
    wo = np.ascontiguousarray(np.asarray(Wo, np.float32))

    in_maps = []
    for c in range(NCORES):
        b, a, own, rows = _core_meta(c)
        gid = rows
        kbias = np.zeros((2, T), np.float32)
        for s in range(2):
            qmax = gid[s * 512:(s + 1) * 512].max()
            kbias[s] = np.where(gid > qmax, NEG, 0.0).astype(np.float32)
        chunksel = np.zeros((64, E), np.float32)
        for lc in range(8):
            chunksel[b * 16 + own[lc], lc] = 1.0
        in_maps.append({
            "xp": np.ascontiguousarray(x[b][rows]),
            "noise": np.ascontiguousarray(noise[b][rows[:NOWN]]),
            "wq": wq, "wk": wk, "wv": wv, "wo": wo,
            "wrout": wrout, "rbias": rbias,
            "we1": we1, "we2": we2,
            "kbias": kbias, "qsel": qsel,
            "chunksel": chunksel,
        })
    return in_maps


_prog = None


def run(trace=False, **inputs):
    global _prog
    if _prog is None:
        _prog = build_program()
    in_maps = _host_inputs(**inputs)
    res = bass_utils.run_bass_kernel_spmd(
        _prog, in_maps, core_ids=list(range(NCORES)), trace=trace)
    out = np.zeros((B, T, D), np.float32)
    for c in range(NCORES):
        b, a, own, rows = _core_meta(c)
        out[b][rows[:NOWN]] = res.results[c]["yout"]
    return out, res


def kernel(**inputs):
    out, _ = run(trace=False, **inputs)
    return out


# revision 13
# speedup vs baseline: 1.2648x; 1.2648x over previous
"""Self-contained Trainium2 Bass kernel for nn_CrossLayerBlock (MoE routing).

8-way token parallelism; uniform SPMD program via per-core token permutation
(own tokens first). Causality = data-driven key-bias rows folded into the score
matmul + 4 universal diagonal mask tiles. Router in true fp32; big matmuls in
float32r; attention probabilities/V in bf16. MoE: global capacity via counts
AllGather + prefix-sum matmuls; kept rows scattered per-expert (indirect DMA),
dense per-expert MLP on compacted buffers, indirect gather back.
"""
import numpy as np
from contextlib import ExitStack

import concourse.bass as bass
import concourse.tile as tile
from concourse import bacc, mybir
from concourse import bass_utils
from concourse.masks import make_identity

B, T, D, H, HS, E = 4, 2048, 768, 12, 64, 8
NCORES = 8
NOWN = 1024
NKC = 16
DC = 6
SE = 192
ZROW = E * SE
LN_EPS = 1e-5
NEG = -30.0

f32 = mybir.dt.float32
f32r = mybir.dt.float32r
bf16 = mybir.dt.bfloat16
i32 = mybir.dt.int32
u32 = mybir.dt.uint32
AF = mybir.ActivationFunctionType
ALU = mybir.AluOpType

KCS0 = [8, 9, 10, 11, 0, 1, 2, 3]
KCS1 = [8, 9, 10, 11, 12, 13, 14, 15, 0, 1, 2, 3, 4, 5, 6, 7]


def _ln(nc, pool, xt, eps_col):
    r = pool.tile([128, 1], f32, tag="ln_r", name="ln_r")
    nc.vector.reduce_sum(r[:], xt[:], axis=mybir.AxisListType.X)
    mu = pool.tile([128, 1], f32, tag="ln_mu", name="ln_mu")
    nc.vector.tensor_scalar_mul(mu[:], r[:], 1.0 / D)
    xc = pool.tile([128, D], f32, tag="ln_xc", name="ln_xc")
    nc.vector.tensor_scalar(xc[:], xt[:], mu[:, :1], None, ALU.subtract)
    sq = pool.tile([128, D], f32, tag="ln_sq", name="ln_sq")
    nc.scalar.square(sq[:], xc[:])
    v = pool.tile([128, 1], f32, tag="ln_v", name="ln_v")
    nc.vector.reduce_sum(v[:], sq[:], axis=mybir.AxisListType.X)
    nc.vector.tensor_scalar_mul(v[:], v[:], 1.0 / D)
    lnv = pool.tile([128, 1], f32, tag="ln_lnv", name="ln_lnv")
    nc.scalar.activation(lnv[:], v[:], AF.Ln, bias=eps_col[:, :1])
    rstd = pool.tile([128, 1], f32, tag="ln_rstd", name="ln_rstd")
    nc.scalar.activation(rstd[:], lnv[:], AF.Exp, scale=-0.5)
    xn = pool.tile([128, D], f32, tag="ln_xn", name="ln_xn")
    nc.vector.tensor_scalar(xn[:], xc[:], rstd[:, :1], None, ALU.mult)
    return xn


def build_program():
    nc = bacc.Bacc("TRN2", target_bir_lowering=False, debug=False,
                   enable_asserts=False, num_devices=NCORES)

    din = {}
    for name, shape, dt in [
        ("xp", [T, D], f32), ("noise", [NOWN, E], f32),
        ("wq", [D, D], f32), ("wk", [D, D], f32), ("wv", [D, D], f32),
        ("wo", [D, D], f32), ("wrout", [D, 17], f32), ("rbias", [1, 17], f32),
        ("we1", [E, D, 4 * D], bf16), ("we2", [E, 4 * D, D], bf16),
        ("kbias", [2, T], f32), ("qsel", [2, NOWN], f32),
        ("chunksel", [64, E], f32),
    ]:
        din[name] = nc.dram_tensor(name, shape, dt, kind="ExternalInput").ap()

    yout = nc.dram_tensor("yout", [NOWN, D], f32, kind="ExternalOutput").ap()
    x1dbg = nc.dram_tensor("x1dbg", [NOWN, D], f32, kind="ExternalOutput").ap()
    rdbg = nc.dram_tensor("rdbg", [NOWN, E], f32, kind="ExternalOutput").ap()

    with tile.TileContext(nc) as tc, ExitStack() as top:
        dram = top.enter_context(tc.tile_pool(name="dram", bufs=1, space="DRAM"))
        xe_dram = dram.tile([E * SE, D], f32)
        ye_dram = dram.tile([E * SE + 1, D], f32)
        cc_in = dram.tile([9, 8], f32)
        cc_out = dram.tile([72, 8], f32, addr_space="Shared")

        const = top.enter_context(tc.tile_pool(name="const", bufs=1))
        ident = const.tile([128, 128], f32)
        make_identity(nc, ident[:])
        lincl = const.tile([128, 128], f32)
        nc.gpsimd.memset(lincl[:], 1.0)
        nc.gpsimd.affine_select(out=lincl[:], in_=lincl[:],
                                compare_op=ALU.is_ge, fill=0.0, base=0,
                                pattern=[[1, 128]], channel_multiplier=-1)
        lstrict = const.tile([128, 128], f32)
        nc.gpsimd.memset(lstrict[:], 1.0)
        nc.gpsimd.affine_select(out=lstrict[:], in_=lstrict[:],
                                compare_op=ALU.is_gt, fill=0.0, base=0,
                                pattern=[[1, 128]], channel_multiplier=-1)
        dm = []
        for d in range(4):
            dmf = const.tile([128, 512], f32, tag=f"dmf_{d}", name=f"dmf_{d}")
            nc.gpsimd.memset(dmf[:], 1.0)
            nc.gpsimd.affine_select(out=dmf[:], in_=dmf[:],
                                    compare_op=ALU.is_ge, fill=0.0,
                                    base=-d * 128, pattern=[[1, 512]],
                                    channel_multiplier=-1)
            dmb = const.tile([128, 512], bf16, tag=f"dmb_{d}", name=f"dmb_{d}")
            nc.vector.tensor_copy(dmb[:], dmf[:])
            dm.append(dmb)
        ones_r = const.tile([128, 1], f32)
        nc.vector.memset(ones_r[:], 1.0)
        ones1r = const.tile([1, 128], f32)
        nc.vector.memset(ones1r[:], 1.0)
        iota8i = const.tile([128, 8], i32)
        nc.gpsimd.iota(iota8i[:], pattern=[[1, 8]], base=0, channel_multiplier=0)
        iota8 = const.tile([128, 8], f32)
        nc.vector.tensor_copy(iota8[:], iota8i[:])
        iotase = const.tile([128, 8], f32)
        nc.vector.tensor_scalar_mul(iotase[:], iota8[:], float(SE))
        eps_col = const.tile([128, 1], f32)
        nc.vector.memset(eps_col[:], LN_EPS)
        rbias_bc = const.tile([128, 17], f32)
        rb1 = const.tile([1, 17], f32)
        nc.sync.dma_start(rb1[:], din["rbias"][:])
        nc.gpsimd.partition_broadcast(rbias_bc[:], rb1[:])
        wrout_sb = const.tile([128, DC, 17], f32)
        nc.sync.dma_start(wrout_sb[:],
                          din["wrout"].rearrange("(a p) n -> p a n", p=128))
        chsel_sb = const.tile([64, E], f32)
        nc.sync.dma_start(chsel_sb[:], din["chunksel"][:])

        x1_t, xn2_t = [], []
        g1_t, g2_t, ns_t, m0_t, m1_t, m_t, maug_t, gf_t = ([] for _ in range(8))
        keep_t, keepr_t, gidx_t = [], [], []

        with ExitStack() as sCF:
            if True:
                pbcp = sCF.enter_context(tc.tile_pool(name="pbcp", bufs=1))
                attT = pbcp.tile([128, DC, NOWN], f32r)
                xn2T = pbcp.tile([128, DC, NOWN], f32)
                with ExitStack() as sAB:
                    pab = sAB.enter_context(tc.tile_pool(name="pab", bufs=1))
                    xnT = pab.tile([128, DC, T], f32r)
                    vaug = pab.tile([128, NKC, H, HS + 1], bf16)

                    # ---------- Phase A ----------
                    with ExitStack() as sA:
                        wvp = sA.enter_context(tc.tile_pool(name="wvp", bufs=1))
                        wv_sb = wvp.tile([128, DC, D], f32r)
                        nc.sync.dma_start(
                            wv_sb[:],
                            din["wv"].rearrange("(a p) n -> p a n",
                                                p=128).bitcast(f32r))
                        apool = sA.enter_context(tc.tile_pool(name="pa_sb",
                                                              bufs=2))
                        aps = sA.enter_context(
                            tc.tile_pool(name="pa_ps", bufs=2, space="PSUM"))
                        for kc in range(NKC):
                            xt = apool.tile([128, D], f32, tag="xt", name="xt")
                            nc.sync.dma_start(
                                xt[:], din["xp"][kc * 128:(kc + 1) * 128, :])
                            xn = _ln(nc, apool, xt, eps_col)
                            for dc in range(DC):
                                tp = aps.tile([128, 128], f32, tag="tp",
                                              name="tp")
                                nc.tensor.transpose(
                                    tp[:], xn[:, dc * 128:(dc + 1) * 128],
                                    ident[:])
                                nc.vector.tensor_copy(
                                    xnT[:, dc, kc * 128:(kc + 1) * 128], tp[:])
                            for nb in range(2):
                                vp = aps.tile([128, 384], f32, tag="vp",
                                              name="vp")
                                for dc in range(DC):
                                    nc.tensor.matmul(
                                        vp[:],
                                        xnT[:, dc, kc * 128:(kc + 1) * 128],
                                        wv_sb[:, dc, nb * 384:(nb + 1) * 384],
                                        start=(dc == 0), stop=(dc == DC - 1))
                                nc.vector.tensor_copy(
                                    vaug[:, kc, nb * 6:(nb + 1) * 6, 0:HS],
                                    vp[:].rearrange("p (h e) -> p h e", e=HS))
                        nc.vector.memset(vaug[:, :, :, HS:HS + 1], 1.0)

                    # ---------- Phase B ----------
                    with ExitStack() as sB:
                        wqk = sB.enter_context(tc.tile_pool(name="wqk", bufs=1))
                        wq_sb = wqk.tile([128, DC, D], f32r, tag="wq_sb",
                                         name="wq_sb")
                        nc.sync.dma_start(
                            wq_sb[:],
                            din["wq"].rearrange("(a p) n -> p a n",
                                                p=128).bitcast(f32r))
                        wk_sb = wqk.tile([128, DC, D], f32r, tag="wk_sb",
                                         name="wk_sb")
                        nc.sync.dma_start(
                            wk_sb[:],
                            din["wk"].rearrange("(a p) n -> p a n",
                                                p=128).bitcast(f32r))
                        kqpool = sB.enter_context(tc.tile_pool(name="pb_kq",
                                                               bufs=1))
                        bpool = sB.enter_context(tc.tile_pool(name="pb_sb",
                                                              bufs=3))
                        bps = sB.enter_context(
                            tc.tile_pool(name="pb_ps", bufs=2, space="PSUM"))
                        atps = sB.enter_context(
                            tc.tile_pool(name="pb_at", bufs=2, space="PSUM"))
                        for h in range(H):
                            kT = kqpool.tile([66, T], f32r, tag="kT", name="kT")
                            nc.sync.dma_start(kT[64:66, :],
                                              din["kbias"][:].bitcast(f32r))
                            for qb in range(4):
                                kp = bps.tile([64, 512], f32, tag="kp",
                                              name="kp")
                                for dc in range(DC):
                                    nc.tensor.matmul(
                                        kp[:],
                                        wk_sb[:, dc, h * 64:(h + 1) * 64],
                                        xnT[:, dc, qb * 512:(qb + 1) * 512],
                                        start=(dc == 0), stop=(dc == DC - 1))
                                nc.vector.tensor_copy(
                                    kT[0:64, qb * 512:(qb + 1) * 512], kp[:])
                            qT = kqpool.tile([66, NOWN], f32r, tag="qT",
                                             name="qT")
                            nc.sync.dma_start(qT[64:66, :],
                                              din["qsel"][:].bitcast(f32r))
                            for qb in range(2):
                                qp = bps.tile([64, 512], f32, tag="kp",
                                              name="qp")
                                for dc in range(DC):
                                    nc.tensor.matmul(
                                        qp[:],
                                        wq_sb[:, dc, h * 64:(h + 1) * 64],
                                        xnT[:, dc, qb * 512:(qb + 1) * 512],
                                        start=(dc == 0), stop=(dc == DC - 1))
                                nc.vector.tensor_copy(
                                    qT[0:64, qb * 512:(qb + 1) * 512], qp[:])

                            for slot, kcs in ((0, KCS0), (1, KCS1)):
                                at = atps.tile([65, 512], f32, tag="at",
                                               name="at")
                                for j, kc in enumerate(kcs):
                                    st = bps.tile([128, 512], f32, tag="st",
                                                  name="st")
                                    nc.tensor.matmul(
                                        st[:], kT[:, kc * 128:(kc + 1) * 128],
                                        qT[:, slot * 512:(slot + 1) * 512],
                                        start=True, stop=True)
                                    exr = bpool.tile([128, 512], bf16,
                                                     tag="exr", name="exr")
                                    nc.scalar.activation(exr[:], st[:], AF.Exp)
                                    if kc < 8 and kc // 4 == slot:
                                        nc.vector.tensor_tensor(
                                            out=exr[:], in0=exr[:],
                                            in1=dm[kc % 4][:], op=ALU.mult)
                                    nc.tensor.matmul(
                                        at[:], vaug[:, kc, h, :], exr[:],
                                        start=(j == 0),
                                        stop=(j == len(kcs) - 1))
                                rec = bpool.tile([1, 512], f32, tag="rec",
                                                 name="rec")
                                nc.vector.reciprocal(rec[:], at[64:65, :])
                                pbt = bpool.tile([64, 512], f32, tag="pbt",
                                                 name="pbt")
                                nc.gpsimd.partition_broadcast(pbt[:], rec[:])
                                nc.vector.tensor_tensor(
                                    out=attT[(h % 2) * 64:(h % 2) * 64 + 64,
                                             h // 2,
                                             slot * 512:(slot + 1) * 512],
                                    in0=at[0:64, :], in1=pbt[:], op=ALU.mult)

                # ---------- Phase C ----------
                cpersist = top.enter_context(tc.tile_pool(name="cpersist",
                                                          bufs=1,
                                                          side="right"))
                with ExitStack() as sC:
                    wop = sC.enter_context(tc.tile_pool(name="wop", bufs=1))
                    wo_sb = wop.tile([128, DC, D], f32r)
                    nc.sync.dma_start(
                        wo_sb[:],
                        din["wo"].rearrange("(a p) n -> p a n",
                                            p=128).bitcast(f32r))
                    ctmp = sC.enter_context(tc.tile_pool(name="ctmp", bufs=2))
                    cps = sC.enter_context(
                        tc.tile_pool(name="pc_ps", bufs=2, space="PSUM"))
                    for tt in range(8):
                        xo = ctmp.tile([128, D], f32, tag="xo", name="xo")
                        nc.sync.dma_start(
                            xo[:], din["xp"][tt * 128:(tt + 1) * 128, :])
                        x1 = cpersist.tile([128, D], f32, tag=f"x1_{tt}",
                                           name=f"x1_{tt}")
                        for nb in range(2):
                            yp = cps.tile([128, 384], f32, tag="yp", name="yp")
                            for dc in range(DC):
                                nc.tensor.matmul(
                                    yp[:],
                                    attT[:, dc, tt * 128:(tt + 1) * 128],
                                    wo_sb[:, dc, nb * 384:(nb + 1) * 384],
                                    start=(dc == 0), stop=(dc == DC - 1))
                            nc.vector.tensor_add(
                                x1[:, nb * 384:(nb + 1) * 384], yp[:],
                                xo[:, nb * 384:(nb + 1) * 384])
                        nc.sync.dma_start(x1dbg[tt * 128:(tt + 1) * 128, :],
                                          x1[:])
                        xn2raw = _ln(nc, ctmp, x1, eps_col)
                        xn2 = cpersist.tile([128, D], f32, tag=f"xn2_{tt}",
                                            name=f"xn2_{tt}")
                        nc.vector.tensor_copy(xn2[:], xn2raw[:])
                        for dc in range(DC):
                            tp = cps.tile([128, 128], f32, tag="tp2",
                                          name="tp2")
                            nc.tensor.transpose(
                                tp[:], xn2[:, dc * 128:(dc + 1) * 128],
                                ident[:])
                            nc.vector.tensor_copy(
                                xn2T[:, dc, tt * 128:(tt + 1) * 128], tp[:])
                        x1_t.append(x1)
                        xn2_t.append(xn2)

            # ---------- Phase D: router ----------
            rpool = top.enter_context(tc.tile_pool(name="rpool", bufs=1,
                                                   side="right"))
            rps = sCF.enter_context(tc.tile_pool(name="pd_ps", bufs=1,
                                                 space="PSUM"))
            cnt_ps = rps.tile([9, 8], f32)
            with ExitStack() as pd:
                dps = pd.enter_context(
                    tc.tile_pool(name="pd_ps2", bufs=2, space="PSUM"))
                dpool = pd.enter_context(tc.tile_pool(name="pd_tmp", bufs=2))
                for tt in range(8):
                    rp = dps.tile([128, 17], f32, tag="rp", name="rp")
                    for dc in range(DC):
                        nc.tensor.matmul(
                            rp[:], xn2T[:, dc, tt * 128:(tt + 1) * 128],
                            wrout_sb[:, dc, :],
                            start=(dc == 0), stop=(dc == DC - 1))
                    rt = dpool.tile([128, 17], f32, tag="rt", name="rt")
                    nc.vector.tensor_add(rt[:], rp[:], rbias_bc[:])
                    z = dpool.tile([128, 8], f32, tag="z", name="z")
                    nc.scalar.activation(z[:], rt[:, 8:16], AF.Exp)
                    zp1 = dpool.tile([128, 8], f32, tag="zp1", name="zp1")
                    nc.vector.tensor_scalar_add(zp1[:], z[:], 1.0)
                    sp0 = dpool.tile([128, 8], f32, tag="sp0", name="sp0")
                    nc.scalar.activation(sp0[:], zp1[:], AF.Ln)
                    en = dpool.tile([128, 8], f32, tag="en", name="en")
                    nc.scalar.activation(en[:], sp0[:], AF.Exp, scale=-1.0)
                    t1 = dpool.tile([128, 8], f32, tag="t1", name="t1")
                    nc.vector.tensor_tensor(out=t1[:], in0=zp1[:], in1=en[:],
                                            op=ALU.mult)
                    nc.vector.tensor_scalar_add(t1[:], t1[:], -1.0)
                    sp = dpool.tile([128, 8], f32, tag="sp", name="sp")
                    nc.vector.tensor_add(sp[:], sp0[:], t1[:])
                    nt = dpool.tile([128, 8], f32, tag="nt", name="nt")
                    nc.sync.dma_start(
                        nt[:], din["noise"][tt * 128:(tt + 1) * 128, :])
                    nm = dpool.tile([128, 8], f32, tag="nm", name="nm")
                    nc.vector.tensor_tensor(out=nm[:], in0=nt[:], in1=sp[:],
                                            op=ALU.mult)
                    noisy = dpool.tile([128, 8], f32, tag="noisy", name="noisy")
                    nc.vector.tensor_add(noisy[:], rt[:, 0:8], nm[:])
                    t8 = dpool.tile([128, 8], f32, tag="t8", name="t8")
                    nc.vector.max(t8[:], noisy[:])
                    ix = dpool.tile([128, 8], u32, tag="ix", name="ix")
                    nc.vector.max_index(ix[:], t8[:], noisy[:])
                    ixf = dpool.tile([128, 8], f32, tag="ixf", name="ixf")
                    nc.vector.tensor_copy(ixf[:], ix[:])
                    dv = dpool.tile([128, 1], f32, tag="dv", name="dv")
                    nc.vector.tensor_sub(dv[:], t8[:, 1:2], t8[:, 0:1])
                    ge = dpool.tile([128, 1], f32, tag="ge", name="ge")
                    nc.scalar.activation(ge[:], dv[:], AF.Exp)
                    gp1 = dpool.tile([128, 1], f32, tag="gp1", name="gp1")
                    nc.vector.tensor_scalar_add(gp1[:], ge[:], 1.0)
                    g1 = rpool.tile([128, 1], f32, tag=f"g1_{tt}",
                                    name=f"g1_{tt}")
                    nc.vector.reciprocal(g1[:], gp1[:])
                    g2 = rpool.tile([128, 1], f32, tag=f"g2_{tt}",
                                    name=f"g2_{tt}")
                    nc.vector.tensor_tensor(out=g2[:], in0=ge[:], in1=g1[:],
                                            op=ALU.mult)
                    ns = rpool.tile([128, 1], f32, tag=f"ns_{tt}",
                                    name=f"ns_{tt}")
                    nc.vector.tensor_scalar(ns[:], rt[:, 16:17], 0.0, None,
                                            ALU.is_le)
                    m0 = rpool.tile([128, 8], f32, tag=f"m0_{tt}",
                                    name=f"m0_{tt}")
                    nc.vector.tensor_scalar(m0[:], iota8[:], ixf[:, 0:1], None,
                                            ALU.is_equal)
                    m1 = rpool.tile([128, 8], f32, tag=f"m1_{tt}",
                                    name=f"m1_{tt}")
                    nc.vector.tensor_scalar(m1[:], iota8[:], ixf[:, 1:2], None,
                                            ALU.is_equal)
                    gf = rpool.tile([128, 8], f32, tag=f"gf_{tt}",
                                    name=f"gf_{tt}")
                    ga = dpool.tile([128, 8], f32, tag="ga", name="ga")
                    nc.vector.tensor_scalar(ga[:], m0[:], g1[:, :1], None,
                                            ALU.mult)
                    gb = dpool.tile([128, 8], f32, tag="gb", name="gb")
                    nc.vector.tensor_scalar(gb[:], m1[:], g2[:, :1], None,
                                            ALU.mult)
                    nc.vector.tensor_add(gf[:], ga[:], gb[:])
                    m = rpool.tile([128, 8], f32, tag=f"m_{tt}", name=f"m_{tt}")
                    nc.vector.tensor_add(m[:], m0[:], m1[:])
                    nc.vector.tensor_scalar_min(m[:], m[:], 1.0)
                    nc.vector.tensor_scalar(m[:], m[:], ns[:, :1], None,
                                            ALU.mult)
                    maug = rpool.tile([128, 9], f32, tag=f"maug_{tt}",
                                      name=f"maug_{tt}")
                    nc.vector.tensor_copy(maug[:, 0:8], m[:])
                    nc.vector.tensor_copy(maug[:, 8:9], ns[:])
                    nc.tensor.matmul(cnt_ps[:, tt:tt + 1], maug[:], ones_r[:],
                                     start=True, stop=True)
                    g1_t.append(g1); g2_t.append(g2); ns_t.append(ns)
                    m0_t.append(m0); m1_t.append(m1); m_t.append(m)
                    maug_t.append(maug); gf_t.append(gf)

            cnt_sb = rpool.tile([9, 8], f32)
            nc.vector.tensor_copy(cnt_sb[:], cnt_ps[:])
            nc.sync.dma_start(cc_in[:], cnt_sb[:])
            nc.gpsimd.collective_compute(
                "AllGather", ALU.bypass, replica_groups=[list(range(NCORES))],
                ins=[cc_in.opt()], outs=[cc_out.opt()])
            cnts_all = rpool.tile([72, 8], f32)
            nc.sync.dma_start(cnts_all[:], cc_out[:])

            # ---------- Phase E ----------
            flat = rpool.tile([9, 64], f32)
            for r in range(NCORES):
                b2, a = r // 2, r % 2
                if a == 0:
                    nc.sync.dma_start(flat[:, b2 * 16:b2 * 16 + 4],
                                      cnts_all[9 * r:9 * r + 9, 0:4])
                    nc.sync.dma_start(flat[:, b2 * 16 + 12:b2 * 16 + 16],
                                      cnts_all[9 * r:9 * r + 9, 4:8])
                else:
                    nc.sync.dma_start(flat[:, b2 * 16 + 4:b2 * 16 + 12],
                                      cnts_all[9 * r:9 * r + 9, 0:8])
            zf = rpool.tile([9, 64], f32)
            nc.vector.memset(zf[:], 0.0)
            incl = rpool.tile([9, 64], f32)
            nc.vector.tensor_tensor_scan(incl[:], flat[:], zf[:], 0.0,
                                         ALU.add, ALU.add)
            excl = rpool.tile([9, 64], f32)
            nc.vector.tensor_sub(excl[:], incl[:], flat[:])
            tot = rpool.tile([1, 1], f32)
            nc.sync.dma_start(tot[:], incl[8:9, 63:64])
            tot_i = rpool.tile([1, 1], i32)
            nc.vector.tensor_copy(tot_i[:], tot[:])
            cap_i = rpool.tile([1, 1], i32)
            nc.vector.tensor_scalar(cap_i[:], tot_i[:], 2, None,
                                    ALU.arith_shift_right)
            capt = rpool.tile([1, 1], f32)
            nc.vector.tensor_copy(capt[:], cap_i[:])
            cap_bc = rpool.tile([128, 1], f32)
            nc.gpsimd.partition_broadcast(cap_bc[:], capt[:])

            exT_ps = rps.tile([64, 9], f32)
            nc.tensor.transpose(exT_ps[:], excl[:, 0:64], ident[0:9, 0:9])
            exT = rpool.tile([64, 9], f32)
            nc.vector.tensor_copy(exT[:], exT_ps[:])
            myo_ps = rps.tile([9, 8], f32)
            nc.tensor.matmul(myo_ps[:], exT[:, 0:9], chsel_sb[:], start=True,
                             stop=True)
            myo = rpool.tile([9, 8], f32)
            nc.vector.tensor_copy(myo[:], myo_ps[:])
            myoT_ps = rps.tile([8, 9], f32)
            nc.tensor.transpose(myoT_ps[:], myo[:], ident[0:9, 0:9])
            myoT = rpool.tile([8, 9], f32)
            nc.vector.tensor_copy(myoT[:], myoT_ps[:])

            # ---------- Phase F ----------
            kcnt_ps = rps.tile([8, 8], f32)
            with ExitStack() as pf:
                fps = pf.enter_context(
                    tc.tile_pool(name="pf_ps", bufs=2, space="PSUM"))
                for tt in range(8):
                    pr = fps.tile([128, 8], f32, tag="pr", name="pr")
                    orow = rpool.tile([1, 8], f32, tag=f"orow_{tt}",
                                      name=f"orow_{tt}")
                    nc.sync.dma_start(orow[:], myoT[tt:tt + 1, 0:8])
                    nc.tensor.matmul(pr[:], ones1r[:], orow[:],
                                     start=True, stop=False)
                    nc.tensor.matmul(pr[:], lincl[:], maug_t[tt][:, 0:8],
                                     start=False, stop=True)
                    keepb = rpool.tile([128, 8], f32, tag=f"kb_{tt}",
                                       name=f"kb_{tt}")
                    nc.vector.tensor_scalar(keepb[:], pr[:], cap_bc[:, :1],
                                            None, ALU.is_le)
                    keep = rpool.tile([128, 8], f32, tag=f"keep_{tt}",
                                      name=f"keep_{tt}")
                    nc.vector.tensor_tensor(out=keep[:], in0=keepb[:],
                                            in1=m_t[tt][:], op=ALU.mult)
                    nc.tensor.matmul(kcnt_ps[:, tt:tt + 1], keep[:],
                                     ones_r[:], start=True, stop=True)
                    kg = rpool.tile([128, 8], f32, tag=f"kg_{tt}",
                                    name=f"kg_{tt}")
                    nc.vector.tensor_tensor(out=kg[:], in0=keep[:],
                                            in1=gf_t[tt][:], op=ALU.mult)
                    nc.sync.dma_start(rdbg[tt * 128:(tt + 1) * 128, :], kg[:])
                    keep_t.append(keep); keepr_t.append(keep)

            kcnt = rpool.tile([8, 8], f32)
            nc.vector.tensor_copy(kcnt[:], kcnt_ps[:])
            zf8 = rpool.tile([8, 8], f32)
            nc.vector.memset(zf8[:], 0.0)
            kincl = rpool.tile([8, 8], f32)
            nc.vector.tensor_tensor_scan(kincl[:], kcnt[:], zf8[:], 0.0,
                                         ALU.add, ALU.add)
            kexcl = rpool.tile([8, 8], f32)
            nc.vector.tensor_sub(kexcl[:], kincl[:], kcnt[:])
            kexT_ps = rps.tile([8, 8], f32)
            nc.tensor.transpose(kexT_ps[:], kexcl[:], ident[0:8, 0:8])
            kexT = rpool.tile([8, 8], f32)
            nc.vector.tensor_copy(kexT[:], kexT_ps[:])

            with ExitStack() as pf2:
                f2ps = pf2.enter_context(
                    tc.tile_pool(name="pf2_ps", bufs=2, space="PSUM"))
                f2p = pf2.enter_context(tc.tile_pool(name="pf2_sb", bufs=2))
                for tt in range(8):
                    p2 = f2ps.tile([128, 8], f32, tag="p2", name="p2")
                    krow = rpool.tile([1, 8], f32, tag=f"krow_{tt}",
                                      name=f"krow_{tt}")
                    nc.sync.dma_start(krow[:], kexT[tt:tt + 1, :])
                    nc.tensor.matmul(p2[:], ones1r[:], krow[:],
                                     start=True, stop=False)
                    nc.tensor.matmul(p2[:], lstrict[:], keepr_t[tt][:],
                                     start=False, stop=True)
                    slotf = f2p.tile([128, 8], f32, tag="slotf", name="slotf")
                    nc.vector.tensor_add(slotf[:], p2[:], iotase[:])
                    gidx = rpool.tile([128, 2], i32, tag=f"gi_{tt}",
                                      name=f"gi_{tt}")
                    for k, mk in ((0, m0_t[tt]), (1, m1_t[tt])):
                        fim = f2p.tile([128, 8], f32, tag="fim", name="fim")
                        nc.vector.tensor_tensor(out=fim[:], in0=slotf[:],
                                                in1=mk[:], op=ALU.mult)
                        fi = f2p.tile([128, 1], f32, tag="fi", name="fi")
                        nc.vector.reduce_sum(fi[:], fim[:],
                                             axis=mybir.AxisListType.X)
                        km = f2p.tile([128, 8], f32, tag="km", name="km")
                        nc.vector.tensor_tensor(out=km[:], in0=mk[:],
                                                in1=keep_t[tt][:], op=ALU.mult)
                        kept = f2p.tile([128, 1], f32, tag="kept", name="kept")
                        nc.vector.reduce_sum(kept[:], km[:],
                                             axis=mybir.AxisListType.X)
                        u = f2p.tile([128, 1], f32, tag="u", name="u")
                        nc.vector.tensor_scalar_add(u[:], kept[:], -1.0)
                        nc.vector.tensor_scalar_mul(u[:], u[:], -70000.0)
                        fis = f2p.tile([128, 1], f32, tag="fis", name="fis")
                        nc.vector.tensor_add(fis[:], fi[:], u[:])
                        fii = f2p.tile([128, 1], i32, tag="fii", name="fii")
                        nc.vector.tensor_copy(fii[:], fis[:])
                        nc.gpsimd.indirect_dma_start(
                            out=xe_dram[:],
                            out_offset=bass.IndirectOffsetOnAxis(
                                ap=fii[:, :1], axis=0),
                            in_=xn2_t[tt][:], in_offset=None,
                            bounds_check=E * SE - 1, oob_is_err=False)
                        gi = f2p.tile([128, 1], f32, tag="gi2", name="gi2")
                        nc.vector.tensor_tensor(out=gi[:], in0=fi[:],
                                                in1=kept[:], op=ALU.mult)
                        w = f2p.tile([128, 1], f32, tag="u2", name="u2")
                        nc.vector.tensor_scalar_add(w[:], kept[:], -1.0)
                        nc.vector.tensor_scalar_mul(w[:], w[:], -float(ZROW))
                        nc.vector.tensor_add(gi[:], gi[:], w[:])
                        nc.vector.tensor_copy(gidx[:, k:k + 1], gi[:])
                    gidx_t.append(gidx)

        # ---------- Phase G: expert MLPs ----------
        zrow = rpool.tile([128, D], f32)
        nc.vector.memset(zrow[:], 0.0)
        nc.sync.dma_start(ye_dram[ZROW:ZROW + 1, :], zrow[0:1, :])
        with ExitStack() as pg:
            gsb = pg.enter_context(tc.tile_pool(name="pg_sb", bufs=2))
            w1p = pg.enter_context(tc.tile_pool(name="pg_w1", bufs=3))
            w2p = pg.enter_context(tc.tile_pool(name="pg_w2", bufs=3))
            hpool = pg.enter_context(tc.tile_pool(name="pg_h", bufs=2))
            gps = pg.enter_context(
                tc.tile_pool(name="pg_ps", bufs=2, space="PSUM"))
            yps = pg.enter_context(
                tc.tile_pool(name="pg_yps", bufs=1, space="PSUM"))
            ROWS = [(0, 128), (128, 64)]
            for e in range(E):
                xeT = gsb.tile([128, DC, SE], bf16, tag="xeT", name="xeT")
                for r0, rn in ROWS:
                    xe = gsb.tile([128, D], f32, tag="xe", name="xe")
                    nc.sync.dma_start(
                        xe[0:rn, :],
                        xe_dram[e * SE + r0:e * SE + r0 + rn, :])
                    for dc in range(DC):
                        tp = gps.tile([128, 128], f32, tag="tp3", name="tp3")
                        nc.tensor.transpose(
                            tp[:, 0:rn], xe[0:rn, dc * 128:(dc + 1) * 128],
                            ident[0:rn, 0:rn])
                        nc.vector.tensor_copy(
                            xeT[:, dc, r0:r0 + rn], tp[:, 0:rn])
                hT = hpool.tile([128, 24, SE], bf16, tag="hT", name="hT")
                for mt in range(24):
                    w1t = w1p.tile([128, DC, 128], bf16, tag="w1t", name="w1t")
                    nc.sync.dma_start(
                        w1t[:],
                        din["we1"][e].rearrange("(a p) n -> p a n", p=128)
                        [:, :, mt * 128:(mt + 1) * 128])
                    hp = gps.tile([128, SE], f32, tag="hp", name="hp")
                    for dc in range(DC):
                        nc.tensor.matmul(hp[:], w1t[:, dc, :], xeT[:, dc, :],
                                         start=(dc == 0), stop=(dc == DC - 1))
                    nc.vector.tensor_scalar_max(hT[:, mt, :], hp[:], 0.0)
                ypl = [yps.tile([128, 384], f32, tag=f"yp_{i}",
                                name=f"ypl_{i}") for i in range(4)]
                for hc in range(24):
                    w2t = w2p.tile([128, D], bf16, tag="w2t", name="w2t")
                    nc.sync.dma_start(
                        w2t[:], din["we2"][e][hc * 128:(hc + 1) * 128, :])
                    for rt2, (r0, rn) in enumerate(ROWS):
                        for nb in range(2):
                            nc.tensor.matmul(
                                ypl[rt2 * 2 + nb][0:rn, :],
                                hT[:, hc, r0:r0 + rn],
                                w2t[:, nb * 384:(nb + 1) * 384],
                                start=(hc == 0), stop=(hc == 23))
                for rt2, (r0, rn) in enumerate(ROWS):
                    ysb = gsb.tile([128, D], f32, tag="ysb", name="ysb")
                    for nb in range(2):
                        nc.scalar.copy(ysb[0:rn, nb * 384:(nb + 1) * 384],
                                       ypl[rt2 * 2 + nb][0:rn, :])
                    nc.sync.dma_start(
                        ye_dram[e * SE + r0:e * SE + r0 + rn, :],
                        ysb[0:rn, :])

        # ---------- Phase H ----------
        with ExitStack() as ph:
            hsb = ph.enter_context(tc.tile_pool(name="ph_sb", bufs=3))
            for tt in range(8):
                yg0 = hsb.tile([128, D], f32, tag="yg0", name="yg0")
                nc.gpsimd.indirect_dma_start(
                    out=yg0[:], out_offset=None, in_=ye_dram[:],
                    in_offset=bass.IndirectOffsetOnAxis(
                        ap=gidx_t[tt][:, 0:1], axis=0))
                yg1 = hsb.tile([128, D], f32, tag="yg1", name="yg1")
                nc.gpsimd.indirect_dma_start(
                    out=yg1[:], out_offset=None, in_=ye_dram[:],
                    in_offset=bass.IndirectOffsetOnAxis(
                        ap=gidx_t[tt][:, 1:2], axis=0))
                u0 = hsb.tile([128, D], f32, tag="u0", name="u0")
                nc.vector.tensor_scalar(u0[:], yg0[:], g1_t[tt][:, :1], None,
                                        ALU.mult)
                u1 = hsb.tile([128, D], f32, tag="u1", name="u1")
                nc.vector.tensor_scalar(u1[:], yg1[:], g2_t[tt][:, :1], None,
                                        ALU.mult)
                upd = hsb.tile([128, D], f32, tag="upd", name="upd")
                nc.vector.tensor_add(upd[:], u0[:], u1[:])
                nc.vector.tensor_sub(upd[:], upd[:], xn2_t[tt][:])
                nc.vector.tensor_scalar(upd[:], upd[:], ns_t[tt][:, :1], None,
                                        ALU.mult)
                nc.vector.tensor_add(upd[:], upd[:], xn2_t[tt][:])
                out = hsb.tile([128, D], f32, tag="out", name="out")
                nc.vector.tensor_add(out[:], x1_t[tt][:], upd[:])
                nc.sync.dma_start(yout[tt * 128:(tt + 1) * 128, :], out[:])

    nc.compile()
    return nc


_OWN = {0: [0, 1, 2, 3, 12, 13, 14, 15], 1: [4, 5, 6, 7, 8, 9, 10, 11]}


def _core_meta(c):
    b, a = c // 2, c % 2
    own = _OWN[a]
    other = [g for g in range(16) if g not in own]
    perm_chunks = own + other
    rows = np.concatenate([np.arange(g * 128, (g + 1) * 128)
                           for g in perm_chunks])
    return b, a, own, rows


def _host_inputs(x, noise, Wq, Wk, Wv, Wo, Wr, br, Wn, bn, Wsk, bsk, We1, We2,
                 **_unused):
    x = np.asarray(x, np.float32)
    noise = np.asarray(noise, np.float32)
    wq = np.ascontiguousarray(
        (np.transpose(np.asarray(Wq), (1, 0, 2)).reshape(D, D)
         * np.float32(D ** -0.5)).astype(np.float32))
    wk = np.ascontiguousarray(
        np.transpose(np.asarray(Wk), (1, 0, 2)).reshape(D, D)
        .astype(np.float32))
    wv = np.ascontiguousarray(
        np.transpose(np.asarray(Wv), (1, 0, 2)).reshape(D, D)
        .astype(np.float32))
    wrout = np.ascontiguousarray(np.concatenate(
        [np.asarray(Wr), np.asarray(Wn), np.asarray(Wsk)], axis=1)
        .astype(np.float32))
    rbias = np.concatenate(
        [np.asarray(br), np.asarray(bn), np.asarray(bsk)])[None, :] \
        .astype(np.float32)
    qsel = np.zeros((2, NOWN), np.float32)
    qsel[0, 0:512] = 1.0
    qsel[1, 512:1024] = 1.0
    import ml_dtypes
    we1 = np.ascontiguousarray(np.asarray(We1).astype(ml_dtypes.bfloat16))
    we2 = np.ascontiguousarray(np.asarray(We2).astype(ml_dtypes.bfloat16))
    wo = np.ascontiguousarray(np.asarray(Wo, np.float32))

    in_maps = []
    for c in range(NCORES):
        b, a, own, rows = _core_meta(c)
        gid = rows
        kbias = np.zeros((2, T), np.float32)
        for s in range(2):
            qmax = gid[s * 512:(s + 1) * 512].max()
            kbias[s] = np.where(gid > qmax, NEG, 0.0).astype(np.float32)
        chunksel = np.zeros((64, E), np.float32)
        for lc in range(8):
            chunksel[b * 16 + own[lc], lc] = 1.0
        in_maps.append({
            "xp": np.ascontiguousarray(x[b][rows]),
            "noise": np.ascontiguousarray(noise[b][rows[:NOWN]]),
            "wq": wq, "wk": wk, "wv": wv, "wo": wo,
            "wrout": wrout, "rbias": rbias,
            "we1": we1, "we2": we2,
            "kbias": kbias, "qsel": qsel,
            "chunksel": chunksel,
        })
    return in_maps


_prog = None


def run(trace=False, **inputs):
    global _prog
    if _prog is None:
        _prog = build_program()
    in_maps = _host_inputs(**inputs)
    res = bass_utils.run_bass_kernel_spmd(
        _prog, in_maps, core_ids=list(range(NCORES)), trace=trace)
    out = np.zeros((B, T, D), np.float32)
    for c in range(NCORES):
        b, a, own, rows = _core_meta(c)
        out[b][rows[:NOWN]] = res.results[c]["yout"]
    return out, res


def kernel(**inputs):
    out, _ = run(trace=False, **inputs)
    return out
